# revision 38
# baseline (speedup 1.0000x reference)
"""Trainium2 Bass kernel for nn_ConvBlock (SepGconv + LayerNorm + GELU MLP).

Computes, for full inputs:
    a   = einsum('bsc,brsd,dc->brc', x, kernel_basis, kernel_W) + conv_bias
    a   = LayerNorm(a) * ln_scale + ln_bias          (over channels, eps=1e-6)
    out = gelu_tanh(a @ W1 + b1) @ W2 + b2

Shapes: B=2, N=1024 (R=S=N), H=64, D=32, WF=4.

Sharding: the (B*R)=2048 output rows split into 8 contiguous shards of 256
rows, one per NeuronCore. Each core reads its kernel_basis shard once
(memory-bound), contracts over all S on-chip, and runs the LN/MLP tail
locally. x / weights are replicated.

Precision/perf strategy: the PE's weight-load port is slow for fp32
(measured ~427 ns per K=128 reload), so the 32 MB/core kernel_basis shard
must stream through the fast moving-operand port in bf16. To keep ~fp32
accuracy both operands are split hi/lo in bf16:
    kb = kbh + kbl,  x = xh + xl,
    a ~= xh.kbh + xl.kbh + xh.kbl   (xl.kbl ~ 2^-18 is dropped)
Each matmul is  psum[c, (r,d)] += x[s,c]^T @ kb[s,(r,d)]  with N=512
(16 rows x 32 d), K=128 s-chunk, M=64 channels; x tiles are the (tiny)
stationary weights. The d-reduction with kernel_W happens on DVE:
multiply by W broadcast, then a free-axis tensor_reduce over d, yielding
aT (64 ch, 256 rows). LayerNorm runs in this transposed space (stats via
a ones-matmul, rsqrt via a DVE-only Newton iteration so ScalarE's LUT
stays pinned on gelu, partition-broadcast via a K=1 matmul), and the MLP
consumes aT directly (h = W1^T @ aT), so no transposes are needed. The
tail is processed in 4 row-quarters whose emission is staggered through
the main loop so all but the last quarter hide under the DMA stream.
"""

import os

import numpy as np

import concourse.bass as bass
import concourse.tile as tile
from concourse import mybir
from concourse.bass_utils import run_bass_kernel_spmd


def _ensure_axon_hooks():
    """bass_utils imports antenv.axon_hooks when trace=True under axon; some
    images ship antenv without that module. Register a functional stand-in
    (driving NTFF capture via libaxon_pjrt.so) so tracing works, degrading
    to hook=None (no trace, run still works) if the .so is unavailable."""
    import sys
    import types

    try:
        import antenv.axon_hooks  # noqa: F401

        return
    except ImportError:
        pass
    try:
        import antenv
    except ImportError:
        antenv = types.ModuleType("antenv")
        sys.modules["antenv"] = antenv

    mod = types.ModuleType("antenv.axon_hooks")
    mod._hook = None

    def set_axon_ntff_profile_hook(h):
        mod._hook = h

    def get_axon_ntff_profile_hook():
        if mod._hook is None:
            try:
                from trn_agent_boot.trn_boot import _ntff_profile_via_ctypes

                so_path = "/opt/axon/libaxon_pjrt.so"
                if os.path.exists(so_path):
                    mod._hook = _ntff_profile_via_ctypes(so_path)
            except Exception:
                mod._hook = None
        return mod._hook

    mod.set_axon_ntff_profile_hook = set_axon_ntff_profile_hook
    mod.get_axon_ntff_profile_hook = get_axon_ntff_profile_hook
    sys.modules["antenv.axon_hooks"] = mod
    antenv.axon_hooks = mod


try:
    _ensure_axon_hooks()
except Exception:
    pass

F32 = mybir.dt.float32
BF16 = mybir.dt.bfloat16

B, N, H, D, WF = 2, 1024, 64, 32, 4
NCORES = 8
ROWS_PER_CORE = (B * N) // NCORES  # 256
RB = 16  # rows per j-block
N_JBLK = ROWS_PER_CORE // RB  # 16
N_KCHUNK = N // 128  # 8 s-chunks of 128
FH = WF * H  # 256
LN_EPS = 1e-6

_NC_CACHE = None
LAST_EXEC_NS = None


def _build_nc(split_waits=True):
    nc = bass.Bass(target_bir_lowering=False)

    kbh = nc.dram_tensor("kbh", [N_JBLK, 128, N_KCHUNK, RB, D], BF16, kind="ExternalInput")
    kbl = nc.dram_tensor("kbl", [N_JBLK, 128, N_KCHUNK, RB, D], BF16, kind="ExternalInput")
    xcp = nc.dram_tensor("xcp", [128, N_KCHUNK, 2 * H], BF16, kind="ExternalInput")
    wb2 = nc.dram_tensor("wb2", [H, RB * D], F32, kind="ExternalInput")
    cbT = nc.dram_tensor("cbT", [H, 1], F32, kind="ExternalInput")
    lnsT = nc.dram_tensor("lnsT", [H, 1], F32, kind="ExternalInput")
    lnbT = nc.dram_tensor("lnbT", [H, 1], F32, kind="ExternalInput")
    w1 = nc.dram_tensor("w1", [H, FH], F32, kind="ExternalInput")
    b1p = nc.dram_tensor("b1p", [128, 2], F32, kind="ExternalInput")
    w2p = nc.dram_tensor("w2p", [128, 2, H], F32, kind="ExternalInput")
    b2_bcast = nc.dram_tensor("b2_bcast", [128, H], F32, kind="ExternalInput")
    out = nc.dram_tensor("out", [ROWS_PER_CORE, H], F32, kind="ExternalOutput")

    with tile.TileContext(nc) as tc:
        with (
            tc.tile_pool(name="consts", bufs=1) as consts,
            tc.tile_pool(name="kbhp", bufs=7) as kbh_pool,
            tc.tile_pool(name="kblp", bufs=7) as kbl_pool,
            tc.tile_pool(name="mwp", bufs=4) as mw_pool,
            tc.tile_pool(name="work", bufs=2) as work,
            tc.tile_pool(name="pmain", bufs=3, space="PSUM") as pmain,
            tc.tile_pool(name="ptail", bufs=1, space="PSUM") as ptail,
            tc.tile_pool(name="pwarm", bufs=1, space="PSUM") as pwarm,
        ):
            # ---- x (gates every matmul) first, then j-block 0 kernel_basis ----
            xc_sb = consts.tile([128, N_KCHUNK, 2 * H], BF16)
            nc.sync.dma_start(out=xc_sb, in_=xcp[:, :, :])

            kb_tiles = {}
            for j0 in range(3):
                kb_tiles[j0] = (
                    kbh_pool.tile([128, N_KCHUNK, RB, D], BF16, name=f"kbh_t{j0}", tag="kbh_t"),
                    kbl_pool.tile([128, N_KCHUNK, RB, D], BF16, name=f"kbl_t{j0}", tag="kbl_t"),
                )
                nc.sync.dma_start(out=kb_tiles[j0][0], in_=kbh[j0, :, :, :, :])
                nc.sync.dma_start(out=kb_tiles[j0][1], in_=kbl[j0, :, :, :, :])

            # ---- PE warm-up: throwaway matmuls on the x tile while the first
            # kernel_basis supertile is still in flight (HAM needs ~3.4us of
            # activity to unthrottle 1.2 -> 2.4 GHz) ----
            ps_warm = pwarm.tile([128, RB * D], F32)
            for w in range(12):
                nc.tensor.matmul(
                    ps_warm,
                    lhsT=xc_sb[:, 0, :],
                    rhs=xc_sb.rearrange("p a b -> p (a b)")[:, 0 : RB * D],
                    start=True,
                    stop=True,
                )

            # ---- remaining constants ----
            wb_sb = consts.tile([H, RB * D], F32)
            nc.sync.dma_start(out=wb_sb, in_=wb2[:, :])
            cb_sb = consts.tile([H, 1], F32)
            nc.sync.dma_start(out=cb_sb, in_=cbT[:, :])
            lns_sb = consts.tile([H, 1], F32)
            nc.sync.dma_start(out=lns_sb, in_=lnsT[:, :])
            lnb_sb = consts.tile([H, 1], F32)
            nc.sync.dma_start(out=lnb_sb, in_=lnbT[:, :])
            w1_sb = consts.tile([H, FH], F32)
            nc.sync.dma_start(out=w1_sb, in_=w1[:, :])
            b1_sb = consts.tile([128, 2], F32)
            nc.sync.dma_start(out=b1_sb, in_=b1p[:, :])
            w2_sb = consts.tile([128, 2, H], F32)
            nc.sync.dma_start(out=w2_sb, in_=w2p[:, :, :])
            b2_sb = consts.tile([128, H], F32)
            nc.sync.dma_start(out=b2_sb, in_=b2_bcast[:, :])
            ones64 = consts.tile([H, 1], F32)
            nc.vector.memset(ones64, 1.0)
            ones1 = consts.tile([1, H], F32)
            nc.vector.memset(ones1, 1.0)
            aT = consts.tile([H, ROWS_PER_CORE], F32)

            # ---- tail pieces, per quarter of rows (64 each), emission
            # staggered through the j-loop so every PE op's inputs are
            # long-ready when the PE reaches it (in-order queues) ----
            Q = ROWS_PER_CORE // 4  # 64
            state = {}

            def t_stacked(q):
                sl = slice(Q * q, Q * (q + 1))
                st = work.tile([H, 2 * Q], F32, name=f"stacked{q}", tag="stacked")
                nc.vector.tensor_scalar(
                    out=st[:, 0:Q], in0=aT[:, sl], scalar1=cb_sb,
                    scalar2=None, op0=mybir.AluOpType.add,
                )
                nc.vector.tensor_mul(st[:, Q : 2 * Q], st[:, 0:Q], st[:, 0:Q])
                state[("st", q)] = st

            def t_stats(q):
                st = state[("st", q)]
                ps_s = ptail.tile([1, 2 * Q], F32, name=f"ps_s{q}", tag="ps_s", bufs=1)
                nc.tensor.matmul(ps_s, lhsT=ones64, rhs=st, start=True, stop=True)
                m = work.tile([1, 2 * Q], F32, name=f"m{q}", tag="m")
                nc.vector.tensor_scalar(
                    out=m, in0=ps_s, scalar1=1.0 / H, scalar2=None,
                    op0=mybir.AluOpType.mult,
                )
                var = work.tile([1, Q], F32, name=f"var{q}", tag="var")
                nc.vector.tensor_mul(var, m[:, 0:Q], m[:, 0:Q])
                nc.vector.tensor_sub(var, m[:, Q : 2 * Q], var)
                qt = work.tile([1, Q], F32, name=f"qt{q}", tag="qt")
                nc.vector.tensor_scalar(
                    out=qt, in0=var, scalar1=LN_EPS, scalar2=None,
                    op0=mybir.AluOpType.add,
                )
                # rsqrt on DVE only (keeps ScalarE's table pinned on gelu):
                # quake seed via int<->float value casts, then 3 Newton steps.
                uf = work.tile([1, Q], F32, name=f"uf{q}", tag="uf")
                nc.vector.tensor_copy(out=uf, in_=qt.bitcast(mybir.dt.int32))
                nc.vector.tensor_scalar(
                    out=uf, in0=uf, scalar1=-0.5, scalar2=float(0x5F3759DF),
                    op0=mybir.AluOpType.mult, op1=mybir.AluOpType.add,
                )
                yi = work.tile([1, Q], mybir.dt.int32, name=f"yi{q}", tag="yi")
                nc.vector.tensor_copy(out=yi, in_=uf)
                y = yi.bitcast(F32)
                t1 = work.tile([1, Q], F32, name=f"t1_{q}", tag="t1")
                for _ in range(2):
                    nc.vector.tensor_mul(t1, y, y)
                    nc.vector.tensor_mul(t1, t1, qt)
                    nc.vector.tensor_scalar(
                        out=t1, in0=t1, scalar1=-0.5, scalar2=1.5,
                        op0=mybir.AluOpType.mult, op1=mybir.AluOpType.add,
                    )
                    nc.vector.tensor_mul(y, y, t1)
                rp = work.tile([1, 2 * Q], F32, name=f"rp{q}", tag="rp")
                nc.vector.tensor_copy(out=rp[:, 0:Q], in_=y)
                nc.vector.tensor_mul(rp[:, Q : 2 * Q], m[:, 0:Q], rp[:, 0:Q])
                state[("rp", q)] = rp

            def t_bc(q):
                rp = state[("rp", q)]
                st = state[("st", q)]
                ps_bc = ptail.tile([H, 2 * Q], F32, name=f"ps_bc{q}", tag="ps_bc", bufs=1)
                nc.tensor.matmul(ps_bc, lhsT=ones1, rhs=rp, start=True, stop=True)
                aln = work.tile([H, Q], F32, name=f"aln{q}", tag="aln")
                nc.vector.tensor_mul(aln, st[:, 0:Q], ps_bc[:, 0:Q])
                nc.vector.tensor_sub(aln, aln, ps_bc[:, Q : 2 * Q])
                nc.vector.tensor_scalar(
                    out=aln, in0=aln, scalar1=lns_sb, scalar2=lnb_sb,
                    op0=mybir.AluOpType.mult, op1=mybir.AluOpType.add,
                )
                state[("aln", q)] = aln

            def t_mlp(q):
                aln = state[("aln", q)]
                hT = work.tile([128, 2, Q], F32, name=f"hT{q}", tag="hT")
                for fh in range(2):
                    ph = ptail.tile([128, Q], F32, name=f"ph{q}_{fh}", tag="ph", bufs=1)
                    nc.tensor.matmul(
                        ph,
                        lhsT=w1_sb[:, 128 * fh : 128 * (fh + 1)],
                        rhs=aln,
                        start=True,
                        stop=True,
                    )
                    nc.scalar.activation(
                        out=hT[:, fh, :],
                        in_=ph,
                        func=mybir.ActivationFunctionType.Gelu_apprx_tanh,
                        bias=b1_sb[:, fh : fh + 1],
                        scale=1.0,
                    )
                po = ptail.tile([Q, H], F32, name=f"po{q}", tag="po", bufs=1)
                for fh in range(2):
                    nc.tensor.matmul(
                        po,
                        lhsT=hT[:, fh, :],
                        rhs=w2_sb[:, fh, :],
                        start=(fh == 0),
                        stop=(fh == 1),
                    )
                o_sb = work.tile([Q, H], F32, name=f"o_sb{q}", tag="o_sb")
                nc.vector.tensor_add(o_sb, po, b2_sb[0:Q, :])
                nc.sync.dma_start(out=out[Q * q : Q * (q + 1), :], in_=o_sb)

            sched = {
                3: [lambda: t_stacked(0)],
                5: [lambda: t_stats(0)],
                7: [lambda: t_bc(0), lambda: t_stacked(1)],
                9: [lambda: t_mlp(0), lambda: t_stats(1)],
                11: [lambda: t_bc(1), lambda: t_stacked(2)],
                13: [lambda: t_mlp(1), lambda: t_stats(2)],
            }

            # ---- main contraction ----
            for j in range(N_JBLK):
                if j in kb_tiles:
                    kbh_t, kbl_t = kb_tiles.pop(j)
                else:
                    kbh_t = kbh_pool.tile([128, N_KCHUNK, RB, D], BF16, name="kbh_t", tag="kbh_t")
                    kbl_t = kbl_pool.tile([128, N_KCHUNK, RB, D], BF16, name="kbl_t", tag="kbl_t")
                    nc.sync.dma_start(out=kbh_t, in_=kbh[j, :, :, :, :])
                    nc.sync.dma_start(out=kbl_t, in_=kbl[j, :, :, :, :])
                ps = pmain.tile([H, RB * D], F32)
                for k in range(N_KCHUNK):
                    nc.tensor.matmul(
                        ps, lhsT=xc_sb[:, k, 0:H], rhs=kbh_t[:, k, :, :],
                        start=(k == 0), stop=False,
                    )
                    nc.tensor.matmul(
                        ps, lhsT=xc_sb[:, k, 0:H], rhs=kbl_t[:, k, :, :],
                        start=False, stop=False,
                    )
                    nc.tensor.matmul(
                        ps, lhsT=xc_sb[:, k, H : 2 * H], rhs=kbh_t[:, k, :, :],
                        start=False, stop=(k == N_KCHUNK - 1),
                    )
                mw = mw_pool.tile([H, RB, D], F32)
                nc.vector.tensor_mul(
                    mw.rearrange("p a b -> p (a b)"), ps, wb_sb
                )
                nc.vector.tensor_reduce(
                    out=aT[:, RB * j : RB * (j + 1)],
                    in_=mw,
                    axis=mybir.AxisListType.X,
                    op=mybir.AluOpType.add,
                )
                for fn in sched.get(j, ()):
                    fn()

            # remaining tail after the stream: quarters 2 (ready) and 3
            t_bc(2)
            t_mlp(2)
            t_stacked(3)
            t_stats(3)
            t_bc(3)
            t_mlp(3)

    if split_waits:
        _split_matmul_waits(nc)
    return nc


def _split_matmul_waits(nc):
    """This walrus build rejects engine instructions carrying more than one
    semaphore wait ("Too many sync wait commands"). Peel all but the last
    wait off onto same-engine NoOps inserted immediately before the
    instruction — NoOps execute in queue order on the same sequencer, so the
    wait semantics are unchanged."""
    f = nc.m.functions[0]
    nop_id = 0
    for blk in f.blocks:
        insts = list(blk.instructions)
        out = []
        changed = False
        for inst in insts:
            si = inst.sync_info
            if (
                si is not None
                and si.on_wait is not None
                and len(si.on_wait) > 1
                and getattr(inst, "engine", None) is not None
            ):
                waits = list(si.on_wait)
                for w in waits[:-1]:
                    nop = mybir.InstNoOp(
                        name=f"I-mmwait-{nop_id}",
                        engine=inst.engine,
                        ins=[],
                        outs=[],
                        sync_info=mybir.SyncInfo(on_wait=[w], on_update=[]),
                    )
                    nop_id += 1
                    out.append(nop)
                inst.sync_info = mybir.SyncInfo(
                    on_wait=[waits[-1]], on_update=list(si.on_update or [])
                )
                changed = True
            out.append(inst)
        if changed:
            blk.instructions = out


def _get_nc():
    global _NC_CACHE
    if _NC_CACHE is None:
        _NC_CACHE = _build_nc()
    return _NC_CACHE


def _prep_shared(kernel_W, conv_bias, ln_scale, ln_bias, W1, b1, W2, b2):
    import ml_dtypes  # noqa: F401

    # wb2[c, r^*D + d] = W[d, c]
    wb2 = np.ascontiguousarray(np.tile(kernel_W.T.astype(np.float32), (1, RB)))
    cbT = np.ascontiguousarray(conv_bias.reshape(H, 1))
    lnsT = np.ascontiguousarray(ln_scale.reshape(H, 1))
    lnbT = np.ascontiguousarray(ln_bias.reshape(H, 1))
    b1p = np.ascontiguousarray(b1.reshape(2, 128).T)
    w2p = np.ascontiguousarray(W2.reshape(2, 128, H).transpose(1, 0, 2))
    b2b = np.ascontiguousarray(np.broadcast_to(b2, (128, H)))
    return dict(
        wb2=wb2, cbT=cbT, lnsT=lnsT, lnbT=lnbT,
        w1=np.ascontiguousarray(W1), b1p=b1p, w2p=w2p, b2_bcast=b2b,
    )


def _split_hi_lo(a):
    import ml_dtypes

    hi = a.astype(ml_dtypes.bfloat16)
    lo = (a - hi.astype(np.float32)).astype(ml_dtypes.bfloat16)
    return hi, lo


def _prep_x(xb):
    # (N, H) -> (128, k, 2H) = [xh | xl], with s = 128*k + p
    xh, xl = _split_hi_lo(xb)
    f = lambda t: t.reshape(N_KCHUNK, 128, H).transpose(1, 0, 2)
    return np.ascontiguousarray(np.concatenate([f(xh), f(xl)], axis=2))


def _prep_kb_shard(shard):
    # shard (256, 1024, 32) -> (j, p, k, r^, d)
    hi, lo = _split_hi_lo(shard)
    f = lambda t: np.ascontiguousarray(
        t.reshape(N_JBLK, RB, N_KCHUNK, 128, D).transpose(0, 3, 2, 1, 4)
    )
    return f(hi), f(lo)


def kernel(
    x,
    kernel_basis,
    kernel_W,
    conv_bias,
    ln_scale,
    ln_bias,
    W1,
    b1,
    W2,
    b2,
):
    global LAST_EXEC_NS
    x = np.ascontiguousarray(np.asarray(x, np.float32))
    kb = np.ascontiguousarray(np.asarray(kernel_basis, np.float32))
    shared = _prep_shared(
        np.asarray(kernel_W, np.float32),
        np.asarray(conv_bias, np.float32),
        np.asarray(ln_scale, np.float32),
        np.asarray(ln_bias, np.float32),
        np.asarray(W1, np.float32),
        np.asarray(b1, np.float32),
        np.asarray(W2, np.float32),
        np.asarray(b2, np.float32),
    )
    xps = [_prep_x(x[b]) for b in range(B)]

    kbf = kb.reshape(B * N, N, D)
    in_maps = []
    for c in range(NCORES):
        hi, lo = _prep_kb_shard(kbf[c * ROWS_PER_CORE : (c + 1) * ROWS_PER_CORE])
        in_maps.append(dict(kbh=hi, kbl=lo, xcp=xps[c // (NCORES // B)], **shared))

    nc = _get_nc()
    trace = bool(os.environ.get("KERNEL_BASS_TRACE"))
    res = run_bass_kernel_spmd(nc, in_maps, core_ids=list(range(NCORES)), trace=trace)
    LAST_EXEC_NS = res.exec_time_ns

    outs = np.concatenate([res.results[c]["out"] for c in range(NCORES)], axis=0)
    return outs.reshape(B, N, H)


# revision 40
# speedup vs baseline: 1.0098x; 1.0098x over previous
"""Trainium2 Bass kernel for nn_ConvBlock (SepGconv + LayerNorm + GELU MLP).

Computes, for full inputs:
    a   = einsum('bsc,brsd,dc->brc', x, kernel_basis, kernel_W) + conv_bias
    a   = LayerNorm(a) * ln_scale + ln_bias          (over channels, eps=1e-6)
    out = gelu_tanh(a @ W1 + b1) @ W2 + b2

Shapes: B=2, N=1024 (R=S=N), H=64, D=32, WF=4.

Sharding: the (B*R)=2048 output rows split into 8 contiguous shards of 256
rows, one per NeuronCore. Each core reads its kernel_basis shard once
(memory-bound), contracts over all S on-chip, and runs the LN/MLP tail
locally. x / weights are replicated.

Precision/perf strategy: the PE's weight-load port is slow for fp32
(measured ~427 ns per K=128 reload), so the 32 MB/core kernel_basis shard
must stream through the fast moving-operand port in bf16. To keep ~fp32
accuracy both operands are split hi/lo in bf16:
    kb = kbh + kbl,  x = xh + xl,
    a ~= xh.kbh + xl.kbh + xh.kbl   (xl.kbl ~ 2^-18 is dropped)
Each matmul is  psum[c, (r,d)] += x[s,c]^T @ kb[s,(r,d)]  with N=512
(16 rows x 32 d), K=128 s-chunk, M=64 channels; x tiles are the (tiny)
stationary weights. The d-reduction with kernel_W happens on DVE:
multiply by W broadcast, then a free-axis tensor_reduce over d, yielding
aT (64 ch, 256 rows). LayerNorm runs in this transposed space (stats via
a ones-matmul, rsqrt via a DVE-only Newton iteration so ScalarE's LUT
stays pinned on gelu, partition-broadcast via a K=1 matmul), and the MLP
consumes aT directly (h = W1^T @ aT), so no transposes are needed. The
tail is processed in 4 row-quarters whose emission is staggered through
the main loop so all but the last quarter hide under the DMA stream.
"""

import os

import numpy as np

import concourse.bass as bass
import concourse.tile as tile
from concourse import mybir
from concourse.bass_utils import run_bass_kernel_spmd


def _ensure_axon_hooks():
    """bass_utils imports antenv.axon_hooks when trace=True under axon; some
    images ship antenv without that module. Register a functional stand-in
    (driving NTFF capture via libaxon_pjrt.so) so tracing works, degrading
    to hook=None (no trace, run still works) if the .so is unavailable."""
    import sys
    import types

    try:
        import antenv.axon_hooks  # noqa: F401

        return
    except ImportError:
        pass
    try:
        import antenv
    except ImportError:
        antenv = types.ModuleType("antenv")
        sys.modules["antenv"] = antenv

    mod = types.ModuleType("antenv.axon_hooks")
    mod._hook = None

    def set_axon_ntff_profile_hook(h):
        mod._hook = h

    def get_axon_ntff_profile_hook():
        if mod._hook is None:
            try:
                from trn_agent_boot.trn_boot import _ntff_profile_via_ctypes

                so_path = "/opt/axon/libaxon_pjrt.so"
                if os.path.exists(so_path):
                    mod._hook = _ntff_profile_via_ctypes(so_path)
            except Exception:
                mod._hook = None
        return mod._hook

    mod.set_axon_ntff_profile_hook = set_axon_ntff_profile_hook
    mod.get_axon_ntff_profile_hook = get_axon_ntff_profile_hook
    sys.modules["antenv.axon_hooks"] = mod
    antenv.axon_hooks = mod


try:
    _ensure_axon_hooks()
except Exception:
    pass

F32 = mybir.dt.float32
BF16 = mybir.dt.bfloat16

B, N, H, D, WF = 2, 1024, 64, 32, 4
NCORES = 8
ROWS_PER_CORE = (B * N) // NCORES  # 256
RB = 16  # rows per j-block
N_JBLK = ROWS_PER_CORE // RB  # 16
N_KCHUNK = N // 128  # 8 s-chunks of 128
FH = WF * H  # 256
LN_EPS = 1e-6

_NC_CACHE = None
LAST_EXEC_NS = None


def _build_nc(split_waits=True):
    nc = bass.Bass(target_bir_lowering=False)

    kbh = nc.dram_tensor("kbh", [N_JBLK, 128, N_KCHUNK, RB, D], BF16, kind="ExternalInput")
    kbl = nc.dram_tensor("kbl", [N_JBLK, 128, N_KCHUNK, RB, D], BF16, kind="ExternalInput")
    xcp = nc.dram_tensor("xcp", [128, N_KCHUNK, 2 * H], BF16, kind="ExternalInput")
    wb2 = nc.dram_tensor("wb2", [H, RB * D], F32, kind="ExternalInput")
    cbT = nc.dram_tensor("cbT", [H, 1], F32, kind="ExternalInput")
    lnsT = nc.dram_tensor("lnsT", [H, 1], F32, kind="ExternalInput")
    lnbT = nc.dram_tensor("lnbT", [H, 1], F32, kind="ExternalInput")
    w1 = nc.dram_tensor("w1", [H, FH], F32, kind="ExternalInput")
    b1p = nc.dram_tensor("b1p", [128, 2], F32, kind="ExternalInput")
    w2p = nc.dram_tensor("w2p", [128, 2, H], F32, kind="ExternalInput")
    b2_bcast = nc.dram_tensor("b2_bcast", [128, H], F32, kind="ExternalInput")
    out = nc.dram_tensor("out", [ROWS_PER_CORE, H], F32, kind="ExternalOutput")

    with tile.TileContext(nc) as tc:
        with (
            tc.tile_pool(name="consts", bufs=1) as consts,
            tc.tile_pool(name="kbhp", bufs=7) as kbh_pool,
            tc.tile_pool(name="kblp", bufs=7) as kbl_pool,
            tc.tile_pool(name="mwp", bufs=4) as mw_pool,
            tc.tile_pool(name="work", bufs=2) as work,
            tc.tile_pool(name="pmain", bufs=3, space="PSUM") as pmain,
            tc.tile_pool(name="ptail", bufs=1, space="PSUM") as ptail,
            tc.tile_pool(name="pwarm", bufs=1, space="PSUM") as pwarm,
        ):
            # ---- x (gates every matmul) first, then j-block 0 kernel_basis ----
            xc_sb = consts.tile([128, N_KCHUNK, 2 * H], BF16)
            nc.sync.dma_start(out=xc_sb, in_=xcp[:, :, :])

            kb_tiles = {}
            for j0 in range(3):
                kb_tiles[j0] = (
                    kbh_pool.tile([128, N_KCHUNK, RB, D], BF16, name=f"kbh_t{j0}", tag="kbh_t"),
                    kbl_pool.tile([128, N_KCHUNK, RB, D], BF16, name=f"kbl_t{j0}", tag="kbl_t"),
                )
                nc.sync.dma_start(out=kb_tiles[j0][0], in_=kbh[j0, :, :, :, :])
                nc.sync.dma_start(out=kb_tiles[j0][1], in_=kbl[j0, :, :, :, :])

            # ---- PE warm-up: throwaway matmuls on the x tile while the first
            # kernel_basis supertile is still in flight (HAM needs ~3.4us of
            # activity to unthrottle 1.2 -> 2.4 GHz) ----
            ps_warm = pwarm.tile([128, RB * D], F32)
            for w in range(12):
                nc.tensor.matmul(
                    ps_warm,
                    lhsT=xc_sb[:, 0, :],
                    rhs=xc_sb.rearrange("p a b -> p (a b)")[:, 0 : RB * D],
                    start=True,
                    stop=True,
                )

            # ---- remaining constants ----
            wb_sb = consts.tile([H, RB * D], F32)
            nc.sync.dma_start(out=wb_sb, in_=wb2[:, :])
            cb_sb = consts.tile([H, 1], F32)
            nc.sync.dma_start(out=cb_sb, in_=cbT[:, :])
            lns_sb = consts.tile([H, 1], F32)
            nc.sync.dma_start(out=lns_sb, in_=lnsT[:, :])
            lnb_sb = consts.tile([H, 1], F32)
            nc.sync.dma_start(out=lnb_sb, in_=lnbT[:, :])
            w1_sb = consts.tile([H, FH], F32)
            nc.sync.dma_start(out=w1_sb, in_=w1[:, :])
            b1_sb = consts.tile([128, 2], F32)
            nc.sync.dma_start(out=b1_sb, in_=b1p[:, :])
            w2_sb = consts.tile([128, 2, H], F32)
            nc.sync.dma_start(out=w2_sb, in_=w2p[:, :, :])
            b2_sb = consts.tile([128, H], F32)
            nc.sync.dma_start(out=b2_sb, in_=b2_bcast[:, :])
            ones64 = consts.tile([H, 1], F32)
            nc.vector.memset(ones64, 1.0)
            ones1 = consts.tile([1, H], F32)
            nc.vector.memset(ones1, 1.0)
            aT = consts.tile([H, ROWS_PER_CORE], F32)

            # ---- tail pieces, per quarter of rows (64 each), emission
            # staggered through the j-loop so every PE op's inputs are
            # long-ready when the PE reaches it (in-order queues) ----
            Q = ROWS_PER_CORE // 4  # 64
            state = {}

            def t_stacked(q):
                sl = slice(Q * q, Q * (q + 1))
                st = work.tile([H, 2 * Q], F32, name=f"stacked{q}", tag="stacked")
                nc.vector.tensor_scalar(
                    out=st[:, 0:Q], in0=aT[:, sl], scalar1=cb_sb,
                    scalar2=None, op0=mybir.AluOpType.add,
                )
                nc.vector.tensor_mul(st[:, Q : 2 * Q], st[:, 0:Q], st[:, 0:Q])
                state[("st", q)] = st

            def t_stats(q):
                st = state[("st", q)]
                ps_s = ptail.tile([1, 2 * Q], F32, name=f"ps_s{q}", tag="ps_s", bufs=1)
                nc.tensor.matmul(ps_s, lhsT=ones64, rhs=st, start=True, stop=True)
                m = work.tile([1, 2 * Q], F32, name=f"m{q}", tag="m")
                nc.vector.tensor_scalar(
                    out=m, in0=ps_s, scalar1=1.0 / H, scalar2=None,
                    op0=mybir.AluOpType.mult,
                )
                var = work.tile([1, Q], F32, name=f"var{q}", tag="var")
                nc.vector.tensor_mul(var, m[:, 0:Q], m[:, 0:Q])
                nc.vector.tensor_sub(var, m[:, Q : 2 * Q], var)
                qt = work.tile([1, Q], F32, name=f"qt{q}", tag="qt")
                nc.vector.tensor_scalar(
                    out=qt, in0=var, scalar1=LN_EPS, scalar2=None,
                    op0=mybir.AluOpType.add,
                )
                # rsqrt on DVE only (keeps ScalarE's table pinned on gelu):
                # quake seed via int<->float value casts, then 3 Newton steps.
                uf = work.tile([1, Q], F32, name=f"uf{q}", tag="uf")
                nc.vector.tensor_copy(out=uf, in_=qt.bitcast(mybir.dt.int32))
                nc.vector.tensor_scalar(
                    out=uf, in0=uf, scalar1=-0.5, scalar2=float(0x5F3759DF),
                    op0=mybir.AluOpType.mult, op1=mybir.AluOpType.add,
                )
                yi = work.tile([1, Q], mybir.dt.int32, name=f"yi{q}", tag="yi")
                nc.vector.tensor_copy(out=yi, in_=uf)
                y = yi.bitcast(F32)
                t1 = work.tile([1, Q], F32, name=f"t1_{q}", tag="t1")
                for _ in range(2):
                    nc.vector.tensor_mul(t1, y, y)
                    nc.vector.tensor_mul(t1, t1, qt)
                    nc.vector.tensor_scalar(
                        out=t1, in0=t1, scalar1=-0.5, scalar2=1.5,
                        op0=mybir.AluOpType.mult, op1=mybir.AluOpType.add,
                    )
                    nc.vector.tensor_mul(y, y, t1)
                rp = work.tile([1, 2 * Q], F32, name=f"rp{q}", tag="rp")
                nc.vector.tensor_copy(out=rp[:, 0:Q], in_=y)
                nc.vector.tensor_mul(rp[:, Q : 2 * Q], m[:, 0:Q], rp[:, 0:Q])
                state[("rp", q)] = rp

            def t_bc(q):
                rp = state[("rp", q)]
                st = state[("st", q)]
                ps_bc = ptail.tile([H, 2 * Q], F32, name=f"ps_bc{q}", tag="ps_bc", bufs=1)
                nc.tensor.matmul(ps_bc, lhsT=ones1, rhs=rp, start=True, stop=True)
                aln = work.tile([H, Q], F32, name=f"aln{q}", tag="aln")
                nc.vector.tensor_mul(aln, st[:, 0:Q], ps_bc[:, 0:Q])
                nc.vector.tensor_sub(aln, aln, ps_bc[:, Q : 2 * Q])
                nc.vector.tensor_scalar(
                    out=aln, in0=aln, scalar1=lns_sb, scalar2=lnb_sb,
                    op0=mybir.AluOpType.mult, op1=mybir.AluOpType.add,
                )
                state[("aln", q)] = aln

            def t_mlp(q):
                aln = state[("aln", q)]
                hT = work.tile([128, 2, Q], F32, name=f"hT{q}", tag="hT")
                for fh in range(2):
                    ph = ptail.tile([128, Q], F32, name=f"ph{q}_{fh}", tag="ph", bufs=1)
                    nc.tensor.matmul(
                        ph,
                        lhsT=w1_sb[:, 128 * fh : 128 * (fh + 1)],
                        rhs=aln,
                        start=True,
                        stop=True,
                    )
                    nc.scalar.activation(
                        out=hT[:, fh, :],
                        in_=ph,
                        func=mybir.ActivationFunctionType.Gelu_apprx_tanh,
                        bias=b1_sb[:, fh : fh + 1],
                        scale=1.0,
                    )
                po = ptail.tile([Q, H], F32, name=f"po{q}", tag="po", bufs=1)
                for fh in range(2):
                    nc.tensor.matmul(
                        po,
                        lhsT=hT[:, fh, :],
                        rhs=w2_sb[:, fh, :],
                        start=(fh == 0),
                        stop=(fh == 1),
                    )
                o_sb = work.tile([Q, H], F32, name=f"o_sb{q}", tag="o_sb")
                nc.vector.tensor_add(o_sb, po, b2_sb[0:Q, :])
                nc.sync.dma_start(out=out[Q * q : Q * (q + 1), :], in_=o_sb)

            sched = {
                3: [lambda: t_stacked(0)],
                5: [lambda: t_stats(0)],
                7: [lambda: t_bc(0), lambda: t_stacked(1)],
                9: [lambda: t_mlp(0), lambda: t_stats(1)],
                11: [lambda: t_bc(1), lambda: t_stacked(2)],
                13: [lambda: t_mlp(1), lambda: t_stats(2)],
            }

            # ---- main contraction ----
            for j in range(N_JBLK):
                if j in kb_tiles:
                    kbh_t, kbl_t = kb_tiles.pop(j)
                else:
                    kbh_t = kbh_pool.tile([128, N_KCHUNK, RB, D], BF16, name="kbh_t", tag="kbh_t")
                    kbl_t = kbl_pool.tile([128, N_KCHUNK, RB, D], BF16, name="kbl_t", tag="kbl_t")
                    nc.sync.dma_start(out=kbh_t, in_=kbh[j, :, :, :, :])
                    nc.sync.dma_start(out=kbl_t, in_=kbl[j, :, :, :, :])
                ps = pmain.tile([H, RB * D], F32)
                for k in range(N_KCHUNK):
                    nc.tensor.matmul(
                        ps, lhsT=xc_sb[:, k, 0:H], rhs=kbh_t[:, k, :, :],
                        start=(k == 0), stop=False,
                    )
                    nc.tensor.matmul(
                        ps, lhsT=xc_sb[:, k, 0:H], rhs=kbl_t[:, k, :, :],
                        start=False, stop=False,
                    )
                    nc.tensor.matmul(
                        ps, lhsT=xc_sb[:, k, H : 2 * H], rhs=kbh_t[:, k, :, :],
                        start=False, stop=(k == N_KCHUNK - 1),
                    )
                mw = mw_pool.tile([H, RB, D], F32)
                nc.vector.tensor_mul(
                    mw.rearrange("p a b -> p (a b)"), ps, wb_sb
                )
                nc.vector.tensor_reduce(
                    out=aT[:, RB * j : RB * (j + 1)],
                    in_=mw,
                    axis=mybir.AxisListType.X,
                    op=mybir.AluOpType.add,
                )
                for fn in sched.get(j, ()):
                    fn()

            # remaining tail after the stream: quarters 2 (ready) and 3
            t_bc(2)
            t_mlp(2)
            t_stacked(3)
            t_stats(3)
            t_bc(3)
            t_mlp(3)

    if split_waits:
        _split_matmul_waits(nc)
    return nc


def _split_matmul_waits(nc):
    """This walrus build rejects engine instructions carrying more than one
    semaphore wait ("Too many sync wait commands"). Peel all but the last
    wait off onto same-engine NoOps inserted immediately before the
    instruction — NoOps execute in queue order on the same sequencer, so the
    wait semantics are unchanged."""
    f = nc.m.functions[0]
    nop_id = 0
    for blk in f.blocks:
        insts = list(blk.instructions)
        out = []
        changed = False
        for inst in insts:
            si = inst.sync_info
            if (
                si is not None
                and si.on_wait is not None
                and len(si.on_wait) > 1
                and getattr(inst, "engine", None) is not None
            ):
                waits = list(si.on_wait)
                for w in waits[:-1]:
                    nop = mybir.InstNoOp(
                        name=f"I-mmwait-{nop_id}",
                        engine=inst.engine,
                        ins=[],
                        outs=[],
                        sync_info=mybir.SyncInfo(on_wait=[w], on_update=[]),
                    )
                    nop_id += 1
                    out.append(nop)
                inst.sync_info = mybir.SyncInfo(
                    on_wait=[waits[-1]], on_update=list(si.on_update or [])
                )
                changed = True
            out.append(inst)
        if changed:
            blk.instructions = out


def _get_nc():
    global _NC_CACHE
    if _NC_CACHE is None:
        _NC_CACHE = _build_nc()
    return _NC_CACHE


def _prep_shared(kernel_W, conv_bias, ln_scale, ln_bias, W1, b1, W2, b2):
    import ml_dtypes  # noqa: F401

    # wb2[c, r^*D + d] = W[d, c]
    wb2 = np.ascontiguousarray(np.tile(kernel_W.T.astype(np.float32), (1, RB)))
    cbT = np.ascontiguousarray(conv_bias.reshape(H, 1))
    lnsT = np.ascontiguousarray(ln_scale.reshape(H, 1))
    lnbT = np.ascontiguousarray(ln_bias.reshape(H, 1))
    b1p = np.ascontiguousarray(b1.reshape(2, 128).T)
    w2p = np.ascontiguousarray(W2.reshape(2, 128, H).transpose(1, 0, 2))
    b2b = np.ascontiguousarray(np.broadcast_to(b2, (128, H)))
    return dict(
        wb2=wb2, cbT=cbT, lnsT=lnsT, lnbT=lnbT,
        w1=np.ascontiguousarray(W1), b1p=b1p, w2p=w2p, b2_bcast=b2b,
    )


def _split_hi_lo(a):
    import ml_dtypes

    hi = a.astype(ml_dtypes.bfloat16)
    lo = (a - hi.astype(np.float32)).astype(ml_dtypes.bfloat16)
    return hi, lo


def _prep_x(xb):
    # (N, H) -> (128, k, 2H) = [xh | xl], with s = 128*k + p
    xh, xl = _split_hi_lo(xb)
    f = lambda t: t.reshape(N_KCHUNK, 128, H).transpose(1, 0, 2)
    return np.ascontiguousarray(np.concatenate([f(xh), f(xl)], axis=2))


def _prep_kb_shard(shard):
    # shard (256, 1024, 32) -> (j, p, k, r^, d)
    hi, lo = _split_hi_lo(shard)
    f = lambda t: np.ascontiguousarray(
        t.reshape(N_JBLK, RB, N_KCHUNK, 128, D).transpose(0, 3, 2, 1, 4)
    )
    return f(hi), f(lo)


def kernel(
    x,
    kernel_basis,
    kernel_W,
    conv_bias,
    ln_scale,
    ln_bias,
    W1,
    b1,
    W2,
    b2,
):
    global LAST_EXEC_NS
    x = np.ascontiguousarray(np.asarray(x, np.float32))
    kb = np.ascontiguousarray(np.asarray(kernel_basis, np.float32))
    shared = _prep_shared(
        np.asarray(kernel_W, np.float32),
        np.asarray(conv_bias, np.float32),
        np.asarray(ln_scale, np.float32),
        np.asarray(ln_bias, np.float32),
        np.asarray(W1, np.float32),
        np.asarray(b1, np.float32),
        np.asarray(W2, np.float32),
        np.asarray(b2, np.float32),
    )
    xps = [_prep_x(x[b]) for b in range(B)]

    kbf = kb.reshape(B * N, N, D)
    in_maps = []
    for c in range(NCORES):
        hi, lo = _prep_kb_shard(kbf[c * ROWS_PER_CORE : (c + 1) * ROWS_PER_CORE])
        in_maps.append(dict(kbh=hi, kbl=lo, xcp=xps[c // (NCORES // B)], **shared))

    nc = _get_nc()
    trace = bool(os.environ.get("KERNEL_BASS_TRACE"))
    res = run_bass_kernel_spmd(nc, in_maps, core_ids=list(range(NCORES)), trace=trace)
    LAST_EXEC_NS = res.exec_time_ns

    outs = np.concatenate([res.results[c]["out"] for c in range(NCORES)], axis=0)
    return outs.reshape(B, N, H)
